# revision 23
# baseline (speedup 1.0000x reference)
"""Multi-head attention (B=2, N=2048, C=1024, H=16) on 8 trn2 NeuronCores.

Sharding: core c = (batch b = c//4, head-group g = c%4); each core computes
4 heads of one batch end-to-end (qkv proj -> attention -> its slice of the
output projection), host sums the 4 per-batch partials and adds bias.

Device-side formulation avoids transposing the softmax matrix:
  - scores are computed transposed: S^T[k, q] = (K @ Q^T) with the head dim
    contracted, keys on PSUM partitions, queries on the free axis
  - softmax skips max-subtraction (S ~ N(0,1), exp can't overflow) so
    exp(S^T) needs no cross-partition reduction
  - V is augmented with a ones column, so U = [V|1]^T @ exp(S^T) yields both
    the unnormalized attention output (rows 0..63) and the softmax
    denominator (row 64) in one accumulation
  - normalization (1/denom along the free axis) uses a tiny DRAM roundtrip
    to transpose the denominator, a 128-lane reciprocal, and a
    partition-broadcast DMA; applied per-head before the output projection
All matmuls run in float32r (~1.6e-4 rel err, full PE rate at N>=256).
"""
import numpy as np

import concourse.bass as bass
import concourse.mybir as mybir
import concourse.tile as tile
from concourse.bass import ds, ts
from concourse.bass_utils import run_bass_kernel_spmd

F32 = mybir.dt.float32
F32R = mybir.dt.float32r

N = 2048          # sequence length
C = 1024          # model dim
HL = 4            # heads per core
D = 64            # head dim
HD = HL * D       # 256: per-core head-state width


def build_program(iters: int = 1, stages: str = "BCD"):
    # stage letters optionally followed by modifiers:
    #   'm' in stages -> B matmuls only (no input DMAs)
    #   'd' in stages -> B input DMAs only (no matmuls)
    #   'n' in stages -> skip softmax normalization chain in C
    nc = bass.Bass()

    xT = nc.dram_tensor("xT", [C, N], F32R, kind="ExternalInput")
    wqT = nc.dram_tensor("wqT", [C, HD], F32R, kind="ExternalInput")
    wkT = nc.dram_tensor("wkT", [C, HD], F32R, kind="ExternalInput")
    wvT = nc.dram_tensor("wvT", [C, HD], F32R, kind="ExternalInput")
    wp = nc.dram_tensor("wp", [HD, C], F32R, kind="ExternalInput")
    out = nc.dram_tensor("out", [N, C], F32, kind="ExternalOutput")

    dscr = nc.dram_tensor("dscr", [HL, N], F32R)   # denominators
    rscr = nc.dram_tensor("rscr", [HL, N], F32R)   # their reciprocals

    CC = C // 128     # 8 c-chunks
    NT = N // 128     # 16 n(k)-tiles
    NJ = N // 512     # 4 q-chunks

    with tile.TileContext(nc) as tc:
        from contextlib import ExitStack

        for _it in range(iters):
         with ExitStack() as outer:
            # pools used from stage B onward; pools for later stages are
            # created after the stage-B pool releases (allocator reserves
            # space in pool-creation order)
            qk_pool = outer.enter_context(tc.tile_pool(name="qk", bufs=1))
            v_pool = outer.enter_context(tc.tile_pool(name="v", bufs=1))
            ones_sb = v_pool.tile([128, 1], F32, tag="ones", name="ones_sb")
            nc.vector.memset(ones_sb, 1.0)
            # per-head tiles, zero-padded to 128 partitions so score
            # matmuls contract K=128 (full-rate operand streaming); rows
            # 64-127 of both sides are zero so they contribute nothing
            qT = [qk_pool.tile([128, N], F32R, tag=f"qT{h}", name=f"qT{h}") for h in range(HL)]
            kT = [qk_pool.tile([128, N], F32R, tag=f"kT{h}", name=f"kT{h}") for h in range(HL)]
            for tl in qT + kT:
                nc.vector.memset(tl[64:128, :].bitcast(mybir.dt.uint32), 0)
            v_sb = [
                v_pool.tile([128, HL, D + 1], F32R, tag=f"v{i}", name=f"v{i}") for i in range(NT)
            ]
            # ---------------- stage B: qkv projections ----------------
            with ExitStack() as stageb:
                xw_pool = stageb.enter_context(tc.tile_pool(name="xw", bufs=1))
                mm_ps = stageb.enter_context(
                    tc.tile_pool(name="mmps", bufs=3, space="PSUM")
                )
                x_sb = [
                    xw_pool.tile([128, N], F32R, tag=f"x{cc}", name=f"x{cc}") for cc in range(CC)
                ]
                wq_sb = [
                    xw_pool.tile([128, HD], F32R, tag=f"wq{cc}", name=f"wq{cc}") for cc in range(CC)
                ]
                wk_sb = [
                    xw_pool.tile([128, HD], F32R, tag=f"wk{cc}", name=f"wk{cc}") for cc in range(CC)
                ]
                wv_sb = [
                    xw_pool.tile([128, HD], F32R, tag=f"wv{cc}", name=f"wv{cc}") for cc in range(CC)
                ]
                if "m" not in stages:
                    for cc in range(CC):
                        nc.sync.dma_start(out=x_sb[cc], in_=xT[ts(cc, 128), :])
                        nc.sync.dma_start(out=wq_sb[cc], in_=wqT[ts(cc, 128), :])
                        nc.sync.dma_start(out=wk_sb[cc], in_=wkT[ts(cc, 128), :])
                        nc.sync.dma_start(out=wv_sb[cc], in_=wvT[ts(cc, 128), :])
                if "d" in stages:
                    continue

                # qT, kT: per-head [128, N]; rows 0-63 = head data
                for dst, w_sb in ((qT, wq_sb), (kT, wk_sb)):
                    for h in range(HL):
                        for j in range(NJ):
                            ps = mm_ps.tile([64, 512], F32, tag="mm", name="mmps")
                            for cc in range(CC):
                                nc.tensor.matmul(
                                    ps,
                                    w_sb[cc][:, ds(h * D, D)],
                                    x_sb[cc][:, ds(j * 512, 512)],
                                    start=(cc == 0),
                                    stop=(cc == CC - 1),
                                )
                            nc.vector.tensor_copy(
                                dst[h][0:64, ds(j * 512, 512)], ps
                            )

                # v: [n-tile i][128, h, 0:64]; col 64 = 1.0
                for i in range(NT):
                    ps = mm_ps.tile([128, HD], F32, tag="mm", name="vps")
                    for cc in range(CC):
                        nc.tensor.matmul(
                            ps,
                            x_sb[cc][:, ts(i, 128)],
                            wv_sb[cc],
                            start=(cc == 0),
                            stop=(cc == CC - 1),
                        )
                    nc.vector.tensor_copy(
                        v_sb[i][:, :, 0:D],
                        ps.rearrange("p (h d) -> p h d", h=HL),
                    )
                    for h in range(HL):
                        nc.gpsimd.tensor_copy(v_sb[i][:, h, D : D + 1], ones_sb)

            # ---------------- stage C: attention ----------------
            if "C" not in stages:
                continue
            u_pool = outer.enter_context(tc.tile_pool(name="u", bufs=1))
            exp_pool = outer.enter_context(tc.tile_pool(name="exps", bufs=5))
            rb_pool = outer.enter_context(tc.tile_pool(name="rb", bufs=2))
            sm_pool = outer.enter_context(tc.tile_pool(name="sm", bufs=1))
            wp_pool = outer.enter_context(tc.tile_pool(name="wpp", bufs=1))
            out_pool = outer.enter_context(tc.tile_pool(name="outp", bufs=3))
            u_sb = [
                u_pool.tile([128, N], F32R, tag=f"u{h}", name=f"u{h}")
                for h in range(HL)
            ]
            for h in range(HL):
                nc.vector.memset(u_sb[h][64:128, :].bitcast(mybir.dt.uint32), 0)
            s_pool = outer.enter_context(tc.tile_pool(name="ssb", bufs=3))
            c_psum = outer.enter_context(ExitStack())
            s_bufs, u_bufs = (3, 1) if "3" in stages else (2, 2)
            s_ps = c_psum.enter_context(
                tc.tile_pool(name="sps", bufs=s_bufs, space="PSUM")
            )
            u_ps = c_psum.enter_context(
                tc.tile_pool(name="ups", bufs=u_bufs, space="PSUM")
            )
            QW = 1024  # q-chunk width (one exp instruction per S tile)
            for h in range(HL):
                for jj in range(N // QW):
                    u_psum = u_ps.tile([65, QW], F32, tag="u", name="ups")
                    pend = None  # software pipeline: PV lags one k-tile
                    for i in range(NT):
                        s_psum = s_ps.tile([128, QW], F32, tag="s", name="sps")
                        for half in range(QW // 512):
                            nc.tensor.matmul(
                                s_psum[:, ds(half * 512, 512)],
                                kT[h][:, ts(i, 128)],
                                qT[h][:, ds(jj * QW + half * 512, 512)],
                                start=True,
                                stop=True,
                            )
                        e_sb = exp_pool.tile([128, QW], F32R, tag="e", name="esb")
                        if "y" in stages:  # timing probe: fake exp on DVE
                            nc.vector.tensor_copy(e_sb, s_psum)
                        elif i % 4 == 0:
                            # balance engines: ACT reads PSUM at ~half rate
                            # but this offloads the DVE bounce 1 time in 4
                            nc.scalar.activation(
                                out=e_sb,
                                in_=s_psum,
                                func=mybir.ActivationFunctionType.Exp,
                            )
                        else:
                            # ACT reads PSUM at ~half rate; bounce S through
                            # SBUF on DVE so ACT exps at full rate and the
                            # PSUM slot frees early
                            s_sb = s_pool.tile([128, QW], F32, tag="ss", name="ssb")
                            nc.vector.tensor_copy(s_sb, s_psum)
                            nc.scalar.activation(
                                out=e_sb,
                                in_=s_sb,
                                func=mybir.ActivationFunctionType.Exp,
                            )
                        if pend is not None:
                            pi, pe = pend
                            for half in range(QW // 512):
                                nc.tensor.matmul(
                                    u_psum[:, ds(half * 512, 512)],
                                    v_sb[pi][:, h, :],
                                    pe[:, ds(half * 512, 512)],
                                    start=(pi == 0),
                                    stop=(pi == NT - 1),
                                )
                        pend = (i, e_sb)
                    pi, pe = pend
                    for half in range(QW // 512):
                        nc.tensor.matmul(
                            u_psum[:, ds(half * 512, 512)],
                            v_sb[pi][:, h, :],
                            pe[:, ds(half * 512, 512)],
                            start=(pi == 0),
                            stop=(pi == NT - 1),
                        )
                    nc.vector.tensor_copy(
                        u_sb[h][0:65, ds(jj * QW, QW)], u_psum
                    )
                if "n" in stages:
                    continue
                # denominator of head h complete
                nc.sync.dma_start(out=dscr[h, :], in_=u_sb[h][64:65, :])
                dT = sm_pool.tile([128, NT], F32R, tag="dT", name="dT")
                nc.sync.dma_start(
                    out=dT, in_=dscr[h, :].rearrange("(t p) -> p t", p=128)
                )
                rT = sm_pool.tile([128, NT], F32R, tag="rT", name="rT")
                with nc.allow_low_precision(
                    reason="f32r carries full fp32 width here"
                ):
                    nc.vector.reciprocal(rT, dT)
                nc.sync.dma_start(
                    out=rscr[h, :].rearrange("(t p) -> p t", p=128), in_=rT
                )
                rb = rb_pool.tile([64, N], F32R, tag="rb", name="rb")
                rsrc = rscr[h, :]
                nc.sync.dma_start(
                    out=rb,
                    in_=bass.AP(
                        tensor=rsrc.tensor,
                        offset=rsrc.offset,
                        ap=[[0, 64]] + list(rsrc.ap),
                    ),
                )
                nc.gpsimd.tensor_mul(u_sb[h][0:64, :], u_sb[h][0:64, :], rb)

            # ---------------- stage D: output projection ----------------
            if "D" not in stages:
                continue
            c_psum.close()
            d_ps = outer.enter_context(
                tc.tile_pool(name="dps", bufs=3, space="PSUM")
            )
            wp_sb = [
                wp_pool.tile([128, C], F32R, tag=f"wp{h}", name=f"wp{h}")
                for h in range(HL)
            ]
            for h in range(HL):
                nc.sync.dma_start(
                    out=wp_sb[h][0:64, :], in_=wp[h * D : (h + 1) * D, :]
                )
                nc.vector.memset(wp_sb[h][64:128, :].bitcast(mybir.dt.uint32), 0)
            for i in range(NT):
                for o in range(2):
                    ps = d_ps.tile([128, 512], F32, tag="mm", name="dpst")
                    for h in range(HL):
                        nc.tensor.matmul(
                            ps,
                            u_sb[h][:, ts(i, 128)],
                            wp_sb[h][:, ds(o * 512, 512)],
                            start=(h == 0),
                            stop=(h == HL - 1),
                        )
                    o_sb = out_pool.tile([128, 512], F32, tag="o")
                    nc.vector.tensor_copy(o_sb, ps)
                    nc.sync.dma_start(
                        out=out[ts(i, 128), ds(o * 512, 512)], in_=o_sb
                    )

    _split_excess_waits(nc)
    return nc


def _split_excess_waits(nc, cap: int = 1, nop_cap: int = 1):
    """walrus in this env encodes few sync-waits per instruction (1 for
    Matmult's LW struct, ~2 elsewhere); move excess onto preceding
    same-engine NoOps (semantically identical: engines are in-order)."""
    for fn in nc.m.functions:
        for bb in fn.blocks:
            out = []
            changed = False
            for inst in bb.instructions:
                icap = 1 if type(inst).__name__ == "InstMatmult" else cap
                si = inst.sync_info
                waits = list(si.on_wait) if si is not None else []
                if len(waits) > icap:
                    excess, keep = waits[:-icap], waits[-icap:]
                    for i in range(0, len(excess), nop_cap):
                        nop = mybir.InstNoOp(
                            name=nc.get_next_instruction_name(), ins=[], outs=[]
                        )
                        nop.engine = inst.engine
                        nop.sync_info = mybir.SyncInfo(
                            on_wait=excess[i : i + nop_cap], on_update=[]
                        )
                        out.append(nop)
                    inst.sync_info = mybir.SyncInfo(
                        on_wait=keep, on_update=list(si.on_update)
                    )
                    changed = True
                out.append(inst)
            if changed:
                bb.instructions = out


_PROGRAM = None


def _get_program():
    global _PROGRAM
    if _PROGRAM is None:
        _PROGRAM = build_program()
    return _PROGRAM


def _make_in_maps(x, w_qkv, w_proj):
    scale = np.float32((C // 16) ** -0.5)  # head_dim^-0.5 = 0.125
    in_maps = []
    for c in range(8):
        b, g = divmod(c, 4)
        sl = slice(g * HD, (g + 1) * HD)
        xT = np.ascontiguousarray(x[b].T)
        wq = np.ascontiguousarray(w_qkv[0 * C : 1 * C][sl].T * scale)
        wk = np.ascontiguousarray(w_qkv[1 * C : 2 * C][sl].T)
        wv = np.ascontiguousarray(w_qkv[2 * C : 3 * C][sl].T)
        wpm = np.ascontiguousarray(w_proj[:, sl].T)
        in_maps.append(
            {"xT": xT, "wqT": wq, "wkT": wk, "wvT": wv, "wp": wpm}
        )
    return in_maps


def kernel(x, w_qkv, w_proj, b_proj, _trace=False, _trace_kwargs=None):
    x = np.asarray(x, dtype=np.float32)
    w_qkv = np.asarray(w_qkv, dtype=np.float32)
    w_proj = np.asarray(w_proj, dtype=np.float32)
    b_proj = np.asarray(b_proj, dtype=np.float32)

    nc = _get_program()
    in_maps = _make_in_maps(x, w_qkv, w_proj)
    kw = {}
    if _trace:
        kw["trace"] = True
        kw.update(_trace_kwargs or {})
    res = run_bass_kernel_spmd(nc, in_maps, core_ids=list(range(8)), **kw)

    parts = [res.results[c]["out"] for c in range(8)]
    out = np.stack(
        [
            parts[0] + parts[1] + parts[2] + parts[3],
            parts[4] + parts[5] + parts[6] + parts[7],
        ]
    )
    out += b_proj
    kernel._last_results = res
    return out


# revision 24
# speedup vs baseline: 1.2615x; 1.2615x over previous
"""Multi-head attention (B=2, N=2048, C=1024, H=16) on 8 trn2 NeuronCores.

Sharding: core c = (batch b = c//4, head-group g = c%4); each core computes
4 heads of one batch end-to-end (qkv proj -> attention -> its slice of the
output projection), host sums the 4 per-batch partials and adds bias.

Device-side formulation avoids transposing the softmax matrix:
  - scores are computed transposed: S^T[k, q] = (K @ Q^T) with the head dim
    contracted, keys on PSUM partitions, queries on the free axis
  - softmax skips max-subtraction (S ~ N(0,1), exp can't overflow) so
    exp(S^T) needs no cross-partition reduction
  - V is augmented with a ones column, so U = [V|1]^T @ exp(S^T) yields both
    the unnormalized attention output (rows 0..63) and the softmax
    denominator (row 64) in one accumulation
  - normalization (1/denom along the free axis) uses a tiny DRAM roundtrip
    to transpose the denominator, a 128-lane reciprocal, and a
    partition-broadcast DMA; applied per-head before the output projection
All matmuls run in float32r (~1.6e-4 rel err, full PE rate at N>=256).
"""
import numpy as np

import concourse.bass as bass
import concourse.mybir as mybir
import concourse.tile as tile
from concourse.bass import ds, ts
from concourse.bass_utils import run_bass_kernel_spmd

F32 = mybir.dt.float32
F32R = mybir.dt.float32r

N = 2048          # sequence length
C = 1024          # model dim
HL = 4            # heads per core
D = 64            # head dim
HD = HL * D       # 256: per-core head-state width


def build_program(iters: int = 1, stages: str = "BCD"):
    # stage letters optionally followed by modifiers:
    #   'm' in stages -> B matmuls only (no input DMAs)
    #   'd' in stages -> B input DMAs only (no matmuls)
    #   'n' in stages -> skip softmax normalization chain in C
    nc = bass.Bass()

    xT = nc.dram_tensor("xT", [C, N], F32R, kind="ExternalInput")
    wqT = nc.dram_tensor("wqT", [C, HD], F32R, kind="ExternalInput")
    wkT = nc.dram_tensor("wkT", [C, HD], F32R, kind="ExternalInput")
    wvT = nc.dram_tensor("wvT", [C, HD], F32R, kind="ExternalInput")
    wp = nc.dram_tensor("wp", [HD, C], F32R, kind="ExternalInput")
    out = nc.dram_tensor("out", [N, C], F32, kind="ExternalOutput")

    dscr = nc.dram_tensor("dscr", [HL, N], F32R)   # denominators
    rscr = nc.dram_tensor("rscr", [HL, N], F32R)   # their reciprocals

    CC = C // 128     # 8 c-chunks
    NT = N // 128     # 16 n(k)-tiles
    NJ = N // 512     # 4 q-chunks

    with tile.TileContext(nc) as tc:
        from contextlib import ExitStack

        for _it in range(iters):
         with ExitStack() as outer:
            # pools used from stage B onward; pools for later stages are
            # created after the stage-B pool releases (allocator reserves
            # space in pool-creation order)
            qk_pool = outer.enter_context(tc.tile_pool(name="qk", bufs=1))
            v_pool = outer.enter_context(tc.tile_pool(name="v", bufs=1))
            ones_sb = v_pool.tile([128, 1], F32, tag="ones", name="ones_sb")
            nc.vector.memset(ones_sb, 1.0)
            # per-head tiles, zero-padded to 128 partitions so score
            # matmuls contract K=128 (full-rate operand streaming); rows
            # 64-127 of both sides are zero so they contribute nothing
            qT = [qk_pool.tile([128, N], F32R, tag=f"qT{h}", name=f"qT{h}") for h in range(HL)]
            kT = [qk_pool.tile([128, N], F32R, tag=f"kT{h}", name=f"kT{h}") for h in range(HL)]
            for tl in qT + kT:
                nc.vector.memset(tl[64:128, :].bitcast(mybir.dt.uint32), 0)
            v_sb = [
                v_pool.tile([128, HL, D + 1], F32R, tag=f"v{i}", name=f"v{i}") for i in range(NT)
            ]
            # ---------------- stage B: qkv projections ----------------
            with ExitStack() as stageb:
                xw_pool = stageb.enter_context(tc.tile_pool(name="xw", bufs=1))
                mm_ps = stageb.enter_context(
                    tc.tile_pool(name="mmps", bufs=3, space="PSUM")
                )
                x_sb = [
                    xw_pool.tile([128, N], F32R, tag=f"x{cc}", name=f"x{cc}") for cc in range(CC)
                ]
                wq_sb = [
                    xw_pool.tile([128, HD], F32R, tag=f"wq{cc}", name=f"wq{cc}") for cc in range(CC)
                ]
                wk_sb = [
                    xw_pool.tile([128, HD], F32R, tag=f"wk{cc}", name=f"wk{cc}") for cc in range(CC)
                ]
                wv_sb = [
                    xw_pool.tile([128, HD], F32R, tag=f"wv{cc}", name=f"wv{cc}") for cc in range(CC)
                ]
                if "m" not in stages:
                    for cc in range(CC):
                        nc.sync.dma_start(out=x_sb[cc], in_=xT[ts(cc, 128), :])
                        nc.sync.dma_start(out=wq_sb[cc], in_=wqT[ts(cc, 128), :])
                        nc.sync.dma_start(out=wk_sb[cc], in_=wkT[ts(cc, 128), :])
                        nc.sync.dma_start(out=wv_sb[cc], in_=wvT[ts(cc, 128), :])
                if "d" in stages:
                    continue

                # qT, kT: per-head [128, N]; rows 0-63 = head data
                for dst, w_sb in ((qT, wq_sb), (kT, wk_sb)):
                    for h in range(HL):
                        for j in range(NJ):
                            ps = mm_ps.tile([64, 512], F32, tag="mm", name="mmps")
                            for cc in range(CC):
                                nc.tensor.matmul(
                                    ps,
                                    w_sb[cc][:, ds(h * D, D)],
                                    x_sb[cc][:, ds(j * 512, 512)],
                                    start=(cc == 0),
                                    stop=(cc == CC - 1),
                                )
                            nc.vector.tensor_copy(
                                dst[h][0:64, ds(j * 512, 512)], ps
                            )

                # v: [n-tile i][128, h, 0:64]; col 64 = 1.0
                for i in range(NT):
                    ps = mm_ps.tile([128, HD], F32, tag="mm", name="vps")
                    for cc in range(CC):
                        nc.tensor.matmul(
                            ps,
                            x_sb[cc][:, ts(i, 128)],
                            wv_sb[cc],
                            start=(cc == 0),
                            stop=(cc == CC - 1),
                        )
                    nc.vector.tensor_copy(
                        v_sb[i][:, :, 0:D],
                        ps.rearrange("p (h d) -> p h d", h=HL),
                    )
                    for h in range(HL):
                        nc.gpsimd.tensor_copy(v_sb[i][:, h, D : D + 1], ones_sb)

            # ---------------- stage C: attention ----------------
            if "C" not in stages:
                continue
            u_pool = outer.enter_context(tc.tile_pool(name="u", bufs=1))
            exp_pool = outer.enter_context(tc.tile_pool(name="exps", bufs=5))
            rb_pool = outer.enter_context(tc.tile_pool(name="rb", bufs=2))
            sm_pool = outer.enter_context(tc.tile_pool(name="sm", bufs=1))
            wp_pool = outer.enter_context(tc.tile_pool(name="wpp", bufs=1))
            out_pool = outer.enter_context(tc.tile_pool(name="outp", bufs=3))
            u_sb = [
                u_pool.tile([128, N], F32R, tag=f"u{h}", name=f"u{h}")
                for h in range(HL)
            ]
            for h in range(HL):
                nc.vector.memset(u_sb[h][64:128, :].bitcast(mybir.dt.uint32), 0)
            s_pool = outer.enter_context(tc.tile_pool(name="ssb", bufs=3))
            c_psum = outer.enter_context(ExitStack())
            s_bufs, u_bufs = (3, 1) if "3" in stages else (2, 2)
            s_ps = c_psum.enter_context(
                tc.tile_pool(name="sps", bufs=s_bufs, space="PSUM")
            )
            u_ps = c_psum.enter_context(
                tc.tile_pool(name="ups", bufs=u_bufs, space="PSUM")
            )
            QW = 1024  # q-chunk width (one exp instruction per S tile)
            for h in range(HL):
                for jj in range(N // QW):
                    u_psum = u_ps.tile([65, QW], F32, tag="u", name="ups")
                    pend = None  # software pipeline: PV lags one k-tile
                    for i in range(NT):
                        s_psum = s_ps.tile([128, QW], F32, tag="s", name="sps")
                        for half in range(QW // 512):
                            nc.tensor.matmul(
                                s_psum[:, ds(half * 512, 512)],
                                kT[h][:, ts(i, 128)],
                                qT[h][:, ds(jj * QW + half * 512, 512)],
                                start=True,
                                stop=True,
                            )
                        e_sb = exp_pool.tile([128, QW], F32R, tag="e", name="esb")
                        if "y" in stages:  # timing probe: fake exp on DVE
                            nc.vector.tensor_copy(e_sb, s_psum)
                        elif i % 6 == 0:
                            # balance engines: ACT reads PSUM at ~half rate
                            # but this offloads the DVE bounce 1 time in 4
                            nc.scalar.activation(
                                out=e_sb,
                                in_=s_psum,
                                func=mybir.ActivationFunctionType.Exp,
                            )
                        else:
                            # ACT reads PSUM at ~half rate; bounce S through
                            # SBUF on DVE so ACT exps at full rate and the
                            # PSUM slot frees early
                            s_sb = s_pool.tile([128, QW], F32, tag="ss", name="ssb")
                            nc.vector.tensor_copy(s_sb, s_psum)
                            nc.scalar.activation(
                                out=e_sb,
                                in_=s_sb,
                                func=mybir.ActivationFunctionType.Exp,
                            )
                        if pend is not None:
                            pi, pe = pend
                            for half in range(QW // 512):
                                nc.tensor.matmul(
                                    u_psum[:, ds(half * 512, 512)],
                                    v_sb[pi][:, h, :],
                                    pe[:, ds(half * 512, 512)],
                                    start=(pi == 0),
                                    stop=(pi == NT - 1),
                                )
                        pend = (i, e_sb)
                    pi, pe = pend
                    for half in range(QW // 512):
                        nc.tensor.matmul(
                            u_psum[:, ds(half * 512, 512)],
                            v_sb[pi][:, h, :],
                            pe[:, ds(half * 512, 512)],
                            start=(pi == 0),
                            stop=(pi == NT - 1),
                        )
                    nc.vector.tensor_copy(
                        u_sb[h][0:65, ds(jj * QW, QW)], u_psum
                    )
                if "n" in stages:
                    continue
                # denominator of head h complete
                nc.sync.dma_start(out=dscr[h, :], in_=u_sb[h][64:65, :])
                dT = sm_pool.tile([128, NT], F32R, tag="dT", name="dT")
                nc.sync.dma_start(
                    out=dT, in_=dscr[h, :].rearrange("(t p) -> p t", p=128)
                )
                rT = sm_pool.tile([128, NT], F32R, tag="rT", name="rT")
                with nc.allow_low_precision(
                    reason="f32r carries full fp32 width here"
                ):
                    nc.vector.reciprocal(rT, dT)
                nc.sync.dma_start(
                    out=rscr[h, :].rearrange("(t p) -> p t", p=128), in_=rT
                )
                rb = rb_pool.tile([64, N], F32R, tag="rb", name="rb")
                rsrc = rscr[h, :]
                nc.sync.dma_start(
                    out=rb,
                    in_=bass.AP(
                        tensor=rsrc.tensor,
                        offset=rsrc.offset,
                        ap=[[0, 64]] + list(rsrc.ap),
                    ),
                )
                nc.gpsimd.tensor_mul(u_sb[h][0:64, :], u_sb[h][0:64, :], rb)

            # ---------------- stage D: output projection ----------------
            if "D" not in stages:
                continue
            c_psum.close()
            d_ps = outer.enter_context(
                tc.tile_pool(name="dps", bufs=3, space="PSUM")
            )
            wp_sb = [
                wp_pool.tile([128, C], F32R, tag=f"wp{h}", name=f"wp{h}")
                for h in range(HL)
            ]
            for h in range(HL):
                nc.sync.dma_start(
                    out=wp_sb[h][0:64, :], in_=wp[h * D : (h + 1) * D, :]
                )
                nc.vector.memset(wp_sb[h][64:128, :].bitcast(mybir.dt.uint32), 0)
            for i in range(NT):
                for o in range(2):
                    ps = d_ps.tile([128, 512], F32, tag="mm", name="dpst")
                    for h in range(HL):
                        nc.tensor.matmul(
                            ps,
                            u_sb[h][:, ts(i, 128)],
                            wp_sb[h][:, ds(o * 512, 512)],
                            start=(h == 0),
                            stop=(h == HL - 1),
                        )
                    o_sb = out_pool.tile([128, 512], F32, tag="o", name="osb")
                    if (i + o) % 2 == 0:
                        nc.vector.tensor_copy(o_sb, ps)
                    else:
                        nc.scalar.copy(o_sb, ps)
                    nc.sync.dma_start(
                        out=out[ts(i, 128), ds(o * 512, 512)], in_=o_sb
                    )

    _split_excess_waits(nc)
    return nc


def _split_excess_waits(nc, cap: int = 1, nop_cap: int = 1):
    """walrus in this env encodes few sync-waits per instruction (1 for
    Matmult's LW struct, ~2 elsewhere); move excess onto preceding
    same-engine NoOps (semantically identical: engines are in-order)."""
    for fn in nc.m.functions:
        for bb in fn.blocks:
            out = []
            changed = False
            for inst in bb.instructions:
                icap = 1 if type(inst).__name__ == "InstMatmult" else cap
                si = inst.sync_info
                waits = list(si.on_wait) if si is not None else []
                if len(waits) > icap:
                    excess, keep = waits[:-icap], waits[-icap:]
                    for i in range(0, len(excess), nop_cap):
                        nop = mybir.InstNoOp(
                            name=nc.get_next_instruction_name(), ins=[], outs=[]
                        )
                        nop.engine = inst.engine
                        nop.sync_info = mybir.SyncInfo(
                            on_wait=excess[i : i + nop_cap], on_update=[]
                        )
                        out.append(nop)
                    inst.sync_info = mybir.SyncInfo(
                        on_wait=keep, on_update=list(si.on_update)
                    )
                    changed = True
                out.append(inst)
            if changed:
                bb.instructions = out


_PROGRAM = None


def _get_program():
    global _PROGRAM
    if _PROGRAM is None:
        _PROGRAM = build_program()
    return _PROGRAM


def _make_in_maps(x, w_qkv, w_proj):
    scale = np.float32((C // 16) ** -0.5)  # head_dim^-0.5 = 0.125
    in_maps = []
    for c in range(8):
        b, g = divmod(c, 4)
        sl = slice(g * HD, (g + 1) * HD)
        xT = np.ascontiguousarray(x[b].T)
        wq = np.ascontiguousarray(w_qkv[0 * C : 1 * C][sl].T * scale)
        wk = np.ascontiguousarray(w_qkv[1 * C : 2 * C][sl].T)
        wv = np.ascontiguousarray(w_qkv[2 * C : 3 * C][sl].T)
        wpm = np.ascontiguousarray(w_proj[:, sl].T)
        in_maps.append(
            {"xT": xT, "wqT": wq, "wkT": wk, "wvT": wv, "wp": wpm}
        )
    return in_maps


def kernel(x, w_qkv, w_proj, b_proj, _trace=False, _trace_kwargs=None):
    x = np.asarray(x, dtype=np.float32)
    w_qkv = np.asarray(w_qkv, dtype=np.float32)
    w_proj = np.asarray(w_proj, dtype=np.float32)
    b_proj = np.asarray(b_proj, dtype=np.float32)

    nc = _get_program()
    in_maps = _make_in_maps(x, w_qkv, w_proj)
    kw = {}
    if _trace:
        kw["trace"] = True
        kw.update(_trace_kwargs or {})
    res = run_bass_kernel_spmd(nc, in_maps, core_ids=list(range(8)), **kw)

    parts = [res.results[c]["out"] for c in range(8)]
    out = np.stack(
        [
            parts[0] + parts[1] + parts[2] + parts[3],
            parts[4] + parts[5] + parts[6] + parts[7],
        ]
    )
    out += b_proj
    kernel._last_results = res
    return out
